# revision 15
# baseline (speedup 1.0000x reference)
"""Trainium2 kernel for nn_CDR_75642964017548.

Computes, for x[B=1024, D=1024] and basis[O=256, D=1024]:
    d1[b,o] = sum_d |x[b,d] - basis[o,d]|           (L1, temperature 1.0)
    d2[b,o] = sqrt(sum_d (x[b,d] - basis[o,d])^2)   (L2, temperature 2.0)
    xd = d1 + 0.5*d2
    out[b,o] = -(xd*(1+ALPHA) - ALPHA*sum_o' xd[b,o'])

Sharding: output/centroid-parallel. Each of the 8 cores gets 32 basis rows
and the full x (replicated). Device computes xd rows per core; host
gathers, applies the (tiny) alpha rowsum correction and transposes.

Device layout: D on partitions (8 chunks of 128), B on the free dim.

L1 rewrites |t| (t = x - c) without an abs op (TRN2 TensorScalar has none):
  DVE rows:  sum|t| = (sx - sc) - 2*sum min(t,0);  min-tile via one fp16
             tensor_scalar (op0=subtract per-partition c, op1=min vs 0).
  ACT rows:  sum|t| = 2*sum relu(t) - (sx - sc);   relu-tile via one
             ScalarE activation (func=Relu, bias=-c per-partition).
The partition-reduction runs on TensorE with "selector" weights
(column at the centroid's slot = -+2), 3-way COLUMN-TILED: consecutive
centroids go to array column-groups 0/1/2 (tile_position=(0,32s)) so
three M=32 matmuls stream concurrently (~2.4x PE ingest). Centroid i
lives at PSUM/device row p = 32*(i%3) + i//3; all per-centroid host
arrays (selectors, -2*basis matmul weights, csq, msc) are permuted to
device rows, and the host inverse-permutes the output.

A K=1 matmul with +-1 weights (pmo) adds the sx row to every centroid
row with the correct sign; msc carries -+sc into the finalize.

L2: ||x-c||^2 = ||x||^2 + ||c||^2 - 2*x.c via M=96 PE matmuls of the
permuted (-2*basis) against x chunks plus a K=1 ones-matmul adding
||x||^2; one ScalarE activation computes sqrt(0.25*psum + 0.25*csq)
= 0.5*d2. Finalize: one scalar_tensor_tensor xd = (d1 + msc) + 0.5*d2.
"""

import numpy as np

B, O, D = 1024, 256, 1024
NCORES = 8
OSH = O // NCORES          # 32 centroids per core
NCHUNK = D // 128          # 8 partition chunks
NBLK = 3                   # PE column-tiling ways
PROWS = 96                 # device rows (3 blocks x 32)
ALPHA = 0.005
ACT_ROWS = frozenset({6, 7, 8, 15, 16, 17, 24, 25, 26})  # produced on ScalarE (relu form)
GPS_ROWS = frozenset()  # GpSimd TS measured 15.5us/tile + port-contention with DVE: unused

_cache = {}


def _prow(i: int) -> int:
    return 32 * (i % NBLK) + i // NBLK


def _build():
    import concourse.bass as bass
    import concourse.bacc as bacc
    import concourse.tile as tile
    from concourse import mybir

    f32 = mybir.dt.float32
    f16 = mybir.dt.float16
    Alu = mybir.AluOpType
    Act = mybir.ActivationFunctionType

    nc = bacc.Bacc(
        "TRN2",
        target_bir_lowering=False,
        debug=False,
        enable_asserts=False,
        num_devices=NCORES,
    )

    # DRAM I/O (flat free-dim layouts; column index = chunk*width + inner)
    xT_d = nc.dram_tensor("xT", [128, NCHUNK * B], f16, kind="ExternalInput").ap()
    bT_d = nc.dram_tensor("bT", [128, NCHUNK * OSH], f32, kind="ExternalInput").ap()
    nbT_d = nc.dram_tensor("nbT", [128, NCHUNK * OSH], f32, kind="ExternalInput").ap()
    bm2_d = nc.dram_tensor("bm2", [128, NCHUNK * PROWS], f16, kind="ExternalInput").ap()
    xsq_d = nc.dram_tensor("xsq", [1, B], f16, kind="ExternalInput").ap()
    sx_d = nc.dram_tensor("sx", [1, B], f16, kind="ExternalInput").ap()
    csq_d = nc.dram_tensor("csq", [PROWS, 1], f32, kind="ExternalInput").ap()
    msc_d = nc.dram_tensor("msc", [PROWS, 1], f32, kind="ExternalInput").ap()
    sel_d = nc.dram_tensor("sel", [128, OSH * OSH], f16, kind="ExternalInput").ap()
    pmo_d = nc.dram_tensor("pmo", [1, PROWS], f16, kind="ExternalInput").ap()
    on96_d = nc.dram_tensor("on96", [1, PROWS], f16, kind="ExternalInput").ap()
    out_d = nc.dram_tensor("xd", [PROWS, B], f32, kind="ExternalOutput").ap()

    NJ = B // 512

    with tile.TileContext(nc) as tc:
        with (
            tc.tile_pool(name="const", bufs=1) as const,
            tc.tile_pool(name="absp", bufs=16) as absp,
            tc.tile_pool(name="fin", bufs=1) as fin,
            tc.tile_pool(name="psum", bufs=1, space="PSUM") as psum,
        ):
            # x chunks spread across four engine DMA queues (a single
            # HWDGE queue tops out ~32 GB/s here and would pace the kernel);
            # chunk 0 first on sync so the pipeline starts as soon as it lands
            chunk_eng = [nc.sync, nc.scalar, nc.sync, nc.gpsimd,
                         nc.sync, nc.scalar, nc.sync, nc.gpsimd]
            xTc = []
            for c in range(NCHUNK):
                t = const.tile([128, B], f16, tag=f"xT{c}")
                chunk_eng[c].dma_start(t[:], xT_d[:, c * B : (c + 1) * B])
                xTc.append(t)
            bT = const.tile([128, NCHUNK * OSH], f32, tag="bT")
            nc.sync.dma_start(bT[:], bT_d[:])
            nbT = const.tile([128, NCHUNK * OSH], f32, tag="nbT")
            nc.gpsimd.dma_start(nbT[:], nbT_d[:])
            sel = const.tile([128, OSH * OSH], f16, tag="sel")
            nc.sync.dma_start(sel[:], sel_d[:])
            bm2 = const.tile([128, NCHUNK * PROWS], f16, tag="bm2")
            nc.gpsimd.dma_start(bm2[:], bm2_d[:])
            xsq = const.tile([1, B], f16, tag="xsq")
            nc.gpsimd.dma_start(xsq[:], xsq_d[:])
            sx = const.tile([1, B], f16, tag="sx")
            nc.gpsimd.dma_start(sx[:], sx_d[:])
            csq = const.tile([PROWS, 1], f32, tag="csq")
            nc.gpsimd.dma_start(csq[:], csq_d[:])
            msc = const.tile([PROWS, 1], f32, tag="msc")
            nc.gpsimd.dma_start(msc[:], msc_d[:])
            pmo = const.tile([1, PROWS], f16, tag="pmo")
            nc.gpsimd.dma_start(pmo[:], pmo_d[:])
            on96 = const.tile([1, PROWS], f16, tag="on96")
            nc.gpsimd.dma_start(on96[:], on96_d[:])

            xc_ps = psum.tile([PROWS, B], f32, tag="xc")
            d1_ps = psum.tile([PROWS, B], f32, tag="d1")

            # ---- L1 part (3-way column-tiled reduction) ----
            # The L2 (-2*x.c) matmuls ride inside the chunk loop so PE can
            # start as soon as chunk 0 lands (they need no producer).
            # c-outer so each chunk sweep interleaves all centroid triplets:
            # consecutive matmuls hit different array column-groups (s = i%3)
            # and stream concurrently; producers (DVE/ACT/GPS) overlap.
            triplets = [tuple(range(g, min(g + NBLK, OSH))) for g in range(0, OSH, NBLK)]
            for c in range(NCHUNK):
                for j in range(NJ):
                    sl = slice(j * 512, (j + 1) * 512)
                    nc.tensor.matmul(
                        xc_ps[:, sl],
                        bm2[:, c * PROWS : (c + 1) * PROWS],
                        xTc[c][:, sl],
                        start=(c == 0),
                        stop=False,
                    )
                for grp in triplets:
                    tiles = []
                    for i in grp:
                        a = absp.tile([128, B], f16, tag="abs")
                        if i in ACT_ROWS:
                            nc.scalar.activation(
                                a[:],
                                xTc[c][:],
                                Act.Relu,
                                bias=nbT[:, c * OSH + i : c * OSH + i + 1],
                                scale=1.0,
                            )
                        else:
                            eng = nc.gpsimd if i in GPS_ROWS else nc.vector
                            eng.tensor_scalar(
                                out=a[:],
                                in0=xTc[c][:],
                                scalar1=bT[:, c * OSH + i : c * OSH + i + 1],
                                scalar2=0.0,
                                op0=Alu.subtract,
                                op1=Alu.min,
                            )
                        tiles.append(a)
                    for j in range(NJ):
                        sl = slice(j * 512, (j + 1) * 512)
                        for t, i in enumerate(grp):
                            s = i % NBLK
                            nc.tensor.matmul(
                                d1_ps[32 * s : 32 * s + 32, sl],
                                sel[:, i * OSH : (i + 1) * OSH],
                                tiles[t][:, sl],
                                start=(c == 0 and i < NBLK),
                                stop=False,
                                tile_position=(0, 32 * s),
                                skip_group_check=True,
                            )
            for j in range(NJ):
                sl = slice(j * 512, (j + 1) * 512)
                nc.tensor.matmul(
                    xc_ps[:, sl], on96[:], xsq[:, sl], start=False, stop=True
                )
                nc.tensor.matmul(
                    d1_ps[:, sl], pmo[:], sx[:, sl], start=False, stop=True,
                    skip_group_check=True,
                )

            # ---- finalize: xd = (d1_ps + msc) + sqrt(0.25*xc_ps + 0.25*csq) ----
            h2 = fin.tile([PROWS, B], f32, tag="h2")
            nc.scalar.activation(h2[:], xc_ps[:], Act.Sqrt, bias=csq[:], scale=0.25)
            xd = fin.tile([PROWS, B], f32, tag="xd")
            nc.vector.scalar_tensor_tensor(
                out=xd[:],
                in0=d1_ps[:],
                scalar=msc[:],
                in1=h2[:],
                op0=Alu.add,
                op1=Alu.add,
            )
            nc.sync.dma_start(out_d[:], xd[:])

    nc.compile()
    return nc


def _consts():
    if "sel" not in _cache:
        sel = np.zeros((128, OSH, OSH), dtype=np.float16)
        pmo = np.zeros((1, PROWS), dtype=np.float16)
        on96 = np.zeros((1, PROWS), dtype=np.float16)
        for i in range(OSH):
            sgn = 1.0 if i in ACT_ROWS else -1.0
            r = i // NBLK
            sel[:, i, r] = 2.0 * sgn
            pmo[0, _prow(i)] = -sgn
            on96[0, _prow(i)] = 1.0
        _cache["sel"] = np.ascontiguousarray(sel.reshape(128, OSH * OSH))
        _cache["pmo"] = pmo
        _cache["on96"] = on96
    return _cache["sel"], _cache["pmo"], _cache["on96"]


def _prep_inputs(x: np.ndarray, basis: np.ndarray):
    """Build the 8 per-core input maps (host-side shard + layout prep)."""
    xT = np.ascontiguousarray(x.T)  # [D, B] f32
    xT16 = (
        xT.astype(np.float16)
        .reshape(NCHUNK, 128, B)
        .transpose(1, 0, 2)
        .reshape(128, NCHUNK * B)
    )
    xT16 = np.ascontiguousarray(xT16)
    xsq16 = (x * x).sum(axis=1, dtype=np.float32).astype(np.float16)[None, :]
    sx16 = x.sum(axis=1, dtype=np.float32).astype(np.float16)[None, :]
    sel, pmo, on96 = _consts()
    prows = np.array([_prow(i) for i in range(OSH)])

    in_maps = []
    for k in range(NCORES):
        bs = basis[k * OSH : (k + 1) * OSH]  # [32, D] f32
        bT = (
            np.ascontiguousarray(bs.T)
            .reshape(NCHUNK, 128, OSH)
            .transpose(1, 0, 2)
            .reshape(128, NCHUNK * OSH)
        )
        bT = np.ascontiguousarray(bT).astype(np.float32)
        nbT = np.ascontiguousarray(-bT)
        # -2*basis at device-row columns, [128, NCHUNK*PROWS]
        bm2 = np.zeros((128, NCHUNK, PROWS), dtype=np.float16)
        bTr = bT.reshape(128, NCHUNK, OSH)
        bm2[:, :, prows] = (-2.0 * bTr).astype(np.float16)
        bm2 = np.ascontiguousarray(bm2.reshape(128, NCHUNK * PROWS))
        csq = np.zeros((PROWS, 1), dtype=np.float32)
        csq[prows, 0] = 0.25 * (bs * bs).sum(axis=1, dtype=np.float32)
        msc = np.zeros((PROWS, 1), dtype=np.float32)
        sc = bs.sum(axis=1, dtype=np.float32)
        for i in range(OSH):
            msc[_prow(i), 0] = sc[i] if i in ACT_ROWS else -sc[i]
        in_maps.append(
            {
                "xT": xT16,
                "bT": bT,
                "nbT": nbT,
                "bm2": bm2,
                "xsq": xsq16,
                "sx": sx16,
                "csq": csq,
                "msc": msc,
                "sel": sel,
                "pmo": pmo,
                "on96": on96,
            }
        )
    return in_maps


def _run(x: np.ndarray, basis: np.ndarray, trace: bool = False):
    from concourse import bass_utils

    if "nc" not in _cache:
        _cache["nc"] = _build()
    nc = _cache["nc"]
    in_maps = _prep_inputs(x, basis)
    res = bass_utils.run_bass_kernel_spmd(
        nc, in_maps, core_ids=list(range(NCORES)), trace=trace
    )
    return res


def _postprocess(xd_parts) -> np.ndarray:
    prows = np.array([_prow(i) for i in range(OSH)])
    xd = np.concatenate([p[prows] for p in xd_parts], axis=0)  # [O, B] f32
    s = xd.sum(axis=0, dtype=np.float32)  # [B]
    out = ALPHA * s[:, None] - (1.0 + ALPHA) * xd.T  # [B, O]
    return np.ascontiguousarray(out.astype(np.float32))


def kernel(x: np.ndarray, basis: np.ndarray) -> np.ndarray:
    res = _run(x, basis, trace=False)
    return _postprocess([r["xd"] for r in res.results])
